# revision 17
# baseline (speedup 1.0000x reference)
"""Trainium2 Bass kernel v2 for batched cosine similarity (retrieval_knn).

sim[s, b] = dot(support[s,b,:], X[b,:]) / (max(||support[s,b]||, eps) * max(||X[b]||, eps))
optionally normalized to (sim + 1) / 2.

Shapes: support [512, 4096, 64] f32, X [4096, 64] f32 -> out [512, 4096] f32.

v2 strategy (vs v1): the host packs the support shard for each core into a
*pair-transposed bf16* DRAM tensor supT [128, NP*S]:
    partition p in [0,64)   holds d=p    of b_even(jp),
    partition p in [64,128) holds d=p-64 of b_odd(jp),
    free axis = (jp, s) with s contiguous.
This removes all on-device PE transposes and PSUM evacuations, and halves
HBM traffic (bf16 instead of f32+cast).  Per pair jp the device does:
    Sq   = rhs * rhs                  (DVE / ACT, alternating)
    dot += xw_jp^T  @ rhs             (TensorE, PSUM 32-row stripe)
    sqn += ones_l^T @ Sq              (TensorE)
Per quad of 128 b: sqv = sqrt(scale*sqn) (ACT), rv = 1/sqv (DVE approx),
sim = dot*rv (+0.5) (DVE), DMA out to outT [BL, S]; the host transposes back.
"""

import numpy as np
import ml_dtypes

BF16 = ml_dtypes.bfloat16

S, B, D = 512, 4096, 64
NCORES = 8
BL = B // NCORES          # 512 batch elements per core
NP = BL // 2              # 256 pairs per core
CH = 16                   # pairs per DMA chunk (16 pairs = 2 MB bf16)

_SUP_BUFS = 6
_SQ_BUFS = 8
_PSUM_BUFS = (3, 3)

_prog_cache = {}


def _build(s_sz, bl_sz, normalize, loop_iters=1, skip=()):
    skip = frozenset(skip)
    from concourse import bacc, mybir
    from concourse.tile import TileContext
    from contextlib import ExitStack, nullcontext

    NPp = bl_sz // 2
    NCHUNK = NPp // CH

    nc = bacc.Bacc("TRN2")
    sup = nc.declare_dram_parameter("supT", [128, NPp * s_sz], mybir.dt.bfloat16, isOutput=False)
    xw = nc.declare_dram_parameter("xw", [128, NPp * 32], mybir.dt.bfloat16, isOutput=False)
    onesw = nc.declare_dram_parameter("onesw", [128, 16 * 32], mybir.dt.bfloat16, isOutput=False)
    onesdr = nc.declare_dram_parameter("onesdr", [128, 8, 2, 32], mybir.dt.float8e4, isOutput=False)
    ones8 = nc.declare_dram_parameter("ones8", [128, 16 * 32], mybir.dt.float8e4, isOutput=False)
    out = nc.declare_dram_parameter("outT", [bl_sz, s_sz], mybir.dt.float32, isOutput=True)

    with TileContext(nc) as tc, ExitStack() as ctx:
        singles = ctx.enter_context(tc.tile_pool(name="singles", bufs=1))
        supp = ctx.enter_context(tc.tile_pool(name="sup", bufs=_SUP_BUFS))
        sqp = ctx.enter_context(tc.tile_pool(name="sqt", bufs=_SQ_BUFS))
        finp = ctx.enter_context(tc.tile_pool(name="fin", bufs=2))
        bD, bS = _PSUM_BUFS
        psDot = ctx.enter_context(tc.tile_pool(name="psDot", bufs=bD, space="PSUM"))
        psSqn = ctx.enter_context(tc.tile_pool(name="psSqn", bufs=bS, space="PSUM"))

        t_xw = singles.tile([128, NPp * 32], mybir.dt.bfloat16)
        nc.sync.dma_start(out=t_xw, in_=xw[:, :])
        t_ones = singles.tile([128, 16 * 32], mybir.dt.bfloat16)
        nc.sync.dma_start(out=t_ones, in_=onesw[:, :])
        t_odr = None
        if "sqdr" in skip:
            t_odr = singles.tile([128, 8, 2, 32], mybir.dt.float8e4)
            nc.sync.dma_start(out=t_odr, in_=onesdr[:, :, :, :])
        if "sqfp8" in skip:
            t_ones8 = singles.tile([128, 16 * 32], mybir.dt.float8e4)
            nc.sync.dma_start(out=t_ones8, in_=ones8[:, :])

        loop_ctx = tc.For_i(0, loop_iters, 1) if loop_iters > 1 else nullcontext()
        ctx.enter_context(loop_ctx)

        chunk_t = {}
        quad_ps = {}

        dma_eng = nc.gpsimd if "dmaswdge" in skip else nc.sync

        def finalize(q):
            dot_ps, sqn_ps = quad_ps.pop(q)
            if "fin" in skip:
                return
            sqv = finp.tile([128, s_sz], mybir.dt.float32, tag="fsq", name=f"fsq{q}")
            nc.scalar.activation(
                sqv, sqn_ps, mybir.ActivationFunctionType.Sqrt,
                scale=4.0 if normalize else 1.0,
            )
            rv = finp.tile([128, s_sz], mybir.dt.float32, tag="frv", name=f"frv{q}")
            nc.vector.reciprocal_approx_fast(out=rv, in_=sqv)
            simv = finp.tile([128, s_sz], mybir.dt.float32, tag="fsim", name=f"fsim{q}")
            nc.vector.tensor_mul(simv, dot_ps, rv)
            if normalize:
                nc.vector.tensor_scalar_add(simv, simv, 0.5)
            nc.sync.dma_start(out=out[q * 128:(q + 1) * 128, :], in_=simv)

        # c-innermost order: per l-step emit the 4 dot mms across the 4 PE
        # column groups back-to-back, then the 4 sqn mms — each matmul's
        # weight load hides behind the other column groups' streams.
        for stepi in range(NPp // 4):
            grp = []
            for i in range(4):
                pp = 4 * stepi + i
                q, r = pp // 64, pp % 64
                l, c = r // 4, r % 4
                ch = pp // CH
                if pp % CH == 0:
                    t = supp.tile([128, CH * s_sz], mybir.dt.bfloat16, tag="sup", name=f"sup{ch}")
                    dma_eng.dma_start(out=t, in_=sup[:, ch * CH * s_sz:(ch + 1) * CH * s_sz])
                    chunk_t[ch] = t
                if r == 0:
                    dot_ps = psDot.tile([128, s_sz], mybir.dt.float32, tag="dotq", name=f"dot{q}")
                    sqn_ps = psSqn.tile([128, s_sz], mybir.dt.float32, tag="sqnq", name=f"sqn{q}")
                    if "fin" not in skip:
                        if "mm" in skip or "dotmm" in skip:
                            nc.vector.memset(dot_ps, 0.0)
                        if "mm" in skip or "sqmm" in skip:
                            nc.vector.memset(sqn_ps, 1.0)
                    quad_ps[q] = (dot_ps, sqn_ps)
                dot_ps, sqn_ps = quad_ps[q]
                rhs = chunk_t[ch][:, (pp % CH) * s_sz:(pp % CH + 1) * s_sz]
                Sq = None
                if not ("sq" in skip and "mm" in skip):
                    Sq = sqp.tile([128, s_sz], mybir.dt.bfloat16, tag="sqt", name=f"sq{pp}")
                if "sq" not in skip:
                    if "sqdve" not in skip and pp % 2 == 1:
                        nc.scalar.activation(Sq, rhs, mybir.ActivationFunctionType.Square)
                    else:
                        nc.vector.tensor_mul(Sq, rhs, rhs)
                if "mm" not in skip and "dotmm" not in skip:
                    nc.tensor.matmul(
                        dot_ps[32 * c:32 * (c + 1), :],
                        lhsT=t_xw[:, pp * 32:(pp + 1) * 32],
                        rhs=rhs,
                        start=(l == 0),
                        stop=(l == 15),
                        tile_position=(0, 32 * c),
                    )
                grp.append((c, l, Sq, sqn_ps))
            if "mm" not in skip and "sqmm" not in skip:
                for (c, l, Sq, sqn_ps) in grp:
                    nc.tensor.matmul(
                        sqn_ps[32 * c:32 * (c + 1), :],
                        lhsT=t_ones[:, l * 32:(l + 1) * 32],
                        rhs=Sq,
                        start=(l == 0),
                        stop=(l == 15),
                        tile_position=(0, 32 * c),
                    )
            if (4 * stepi + 3) % 64 == 63:
                finalize((4 * stepi) // 64)

    nc.finalize()
    return nc


def _pair_indices(bl_sz):
    """c-innermost pair order: consecutive matmuls hit different PE column
    groups, so their weight loads hide behind other groups' streams."""
    jp = np.arange(bl_sz // 2)
    q, r = jp // 64, jp % 64
    l, c = r // 4, r % 4
    b0 = q * 128 + 32 * c + 2 * l
    return b0, b0 + 1, l


def _pack_support(support_set):
    """Per-core pair-transposed bf16 pack: [128, NP*S]."""
    sup = np.asarray(support_set, np.float32)
    b0, b1, _ = _pair_indices(BL)
    cores = []
    for k in range(NCORES):
        sc = sup[:, k * BL:(k + 1) * BL, :]              # [S, BL, D]
        g0 = sc[:, b0, :].astype(BF16)                   # [S, NP, D]
        g1 = sc[:, b1, :].astype(BF16)
        # -> [2, D, NP, S] -> [128, NP*S]
        arr = np.stack([g0, g1], axis=0).transpose(0, 3, 2, 1)
        cores.append(np.ascontiguousarray(arr).reshape(128, NP * S))
    return cores


def _pack_host_inputs(x_hat, bl_sz):
    """Fold 1/max(||x||,eps) into X, pack per-core zero-padded bf16 lhsT mats."""
    x = np.asarray(x_hat, np.float32)
    xnorm = np.sqrt((x * x).sum(axis=1, keepdims=True))
    xn = (x / np.maximum(xnorm, 1e-10)).astype(BF16)

    ncores = x.shape[0] // bl_sz
    np_pairs = bl_sz // 2
    b0, b1, ls = _pair_indices(bl_sz)
    xw_cores = []
    for k in range(ncores):
        xw = np.zeros((128, np_pairs * 32), dtype=BF16)
        for jp in range(np_pairs):
            l = int(ls[jp])
            col = jp * 32
            xw[0:64, col + 2 * l] = xn[k * bl_sz + b0[jp]]
            xw[64:128, col + 2 * l + 1] = xn[k * bl_sz + b1[jp]]
        xw_cores.append(xw)

    onesw = np.zeros((128, 16 * 32), dtype=BF16)
    for l in range(16):
        onesw[0:64, l * 32 + 2 * l] = BF16(1.0)
        onesw[64:128, l * 32 + 2 * l + 1] = BF16(1.0)
    return xw_cores, onesw


FP8 = ml_dtypes.float8_e4m3


def _pack_ones_dr():
    """[Ki=128, t=8, Ko=2, M=32] fp8 weights: mm t covers pairs (l=2t, 2t+1);
    out rows 4t..4t+3 <- (ko=0, ki<64), (ko=0, ki>=64), (ko=1, ki<64), (ko=1, ki>=64)."""
    arr = np.zeros((128, 8, 2, 32), dtype=FP8)
    for t in range(8):
        arr[0:64, t, 0, 4 * t] = FP8(1.0)
        arr[64:128, t, 0, 4 * t + 1] = FP8(1.0)
        arr[0:64, t, 1, 4 * t + 2] = FP8(1.0)
        arr[64:128, t, 1, 4 * t + 3] = FP8(1.0)
    return arr


def _get_program(normalize):
    key = (S, BL, bool(normalize))
    if key not in _prog_cache:
        _prog_cache[key] = _build(S, BL, bool(normalize))
    return _prog_cache[key]


def _make_in_maps(support_set, X_hat):
    sup_cores = _pack_support(support_set)
    xw_cores, onesw = _pack_host_inputs(np.asarray(X_hat, np.float32), BL)
    onesdr = _pack_ones_dr()
    ones8 = onesw.astype(FP8)
    return [
        {"supT": sup_cores[k], "xw": xw_cores[k], "onesw": onesw,
         "onesdr": onesdr, "ones8": ones8}
        for k in range(NCORES)
    ]


def _run(support_set, X_hat, normalize, **spmd_kwargs):
    nrm = bool(np.asarray(normalize).item())
    from concourse.bass_utils import run_bass_kernel_spmd

    nc = _get_program(nrm)
    in_maps = _make_in_maps(support_set, X_hat)
    res = run_bass_kernel_spmd(nc, in_maps, list(range(NCORES)), **spmd_kwargs)
    out = np.empty((S, B), dtype=np.float32)
    for k in range(NCORES):
        out[:, k * BL:(k + 1) * BL] = np.asarray(res.results[k]["outT"]).T
    return out, res


def kernel(support_set, X_hat, normalize):
    out, _ = _run(support_set, X_hat, normalize)
    return out


# revision 20
# speedup vs baseline: 1.0142x; 1.0142x over previous
"""Trainium2 Bass kernel v2 for batched cosine similarity (retrieval_knn).

sim[s, b] = dot(support[s,b,:], X[b,:]) / (max(||support[s,b]||, eps) * max(||X[b]||, eps))
optionally normalized to (sim + 1) / 2.

Shapes: support [512, 4096, 64] f32, X [4096, 64] f32 -> out [512, 4096] f32.

v2 strategy (vs v1): the host packs the support shard for each core into a
*pair-transposed bf16* DRAM tensor supT [128, NP*S]:
    partition p in [0,64)   holds d=p    of b_even(jp),
    partition p in [64,128) holds d=p-64 of b_odd(jp),
    free axis = (jp, s) with s contiguous.
This removes all on-device PE transposes and PSUM evacuations, and halves
HBM traffic (bf16 instead of f32+cast).  Per pair jp the device does:
    Sq   = rhs * rhs                  (DVE / ACT, alternating)
    dot += xw_jp^T  @ rhs             (TensorE, PSUM 32-row stripe)
    sqn += ones_l^T @ Sq              (TensorE)
Per quad of 128 b: sqv = sqrt(scale*sqn) (ACT), rv = 1/sqv (DVE approx),
sim = dot*rv (+0.5) (DVE), DMA out to outT [BL, S]; the host transposes back.
"""

import numpy as np
import ml_dtypes

BF16 = ml_dtypes.bfloat16

S, B, D = 512, 4096, 64
NCORES = 8
BL = B // NCORES          # 512 batch elements per core
NP = BL // 2              # 256 pairs per core
CH = 16                   # pairs per DMA chunk (16 pairs = 2 MB bf16)

_SUP_BUFS = 8
_SQ_BUFS = 8
_PSUM_BUFS = (4, 4)

_prog_cache = {}


def _build(s_sz, bl_sz, normalize, loop_iters=1, skip=()):
    skip = frozenset(skip)
    from concourse import bacc, mybir
    from concourse.tile import TileContext
    from contextlib import ExitStack, nullcontext

    NPp = bl_sz // 2
    NCHUNK = NPp // CH

    nc = bacc.Bacc("TRN2")
    sup = nc.declare_dram_parameter("supT", [128, NPp * s_sz], mybir.dt.bfloat16, isOutput=False)
    xw = nc.declare_dram_parameter("xw", [128, NPp * 32], mybir.dt.bfloat16, isOutput=False)
    onesw = nc.declare_dram_parameter("onesw", [128, 16 * 32], mybir.dt.bfloat16, isOutput=False)
    onesdr = nc.declare_dram_parameter("onesdr", [128, 8, 2, 32], mybir.dt.float8e4, isOutput=False)
    ones8 = nc.declare_dram_parameter("ones8", [128, 16 * 32], mybir.dt.float8e4, isOutput=False)
    out = nc.declare_dram_parameter("outT", [bl_sz, s_sz], mybir.dt.float32, isOutput=True)

    with TileContext(nc) as tc, ExitStack() as ctx:
        singles = ctx.enter_context(tc.tile_pool(name="singles", bufs=1))
        supp = ctx.enter_context(tc.tile_pool(name="sup", bufs=_SUP_BUFS))
        sqp = ctx.enter_context(tc.tile_pool(name="sqt", bufs=_SQ_BUFS))
        finp = ctx.enter_context(tc.tile_pool(name="fin", bufs=2))
        bD, bS = _PSUM_BUFS
        psDot = ctx.enter_context(tc.tile_pool(name="psDot", bufs=bD, space="PSUM"))
        psSqn = ctx.enter_context(tc.tile_pool(name="psSqn", bufs=bS, space="PSUM"))

        t_xw = singles.tile([128, NPp * 32], mybir.dt.bfloat16)
        nc.sync.dma_start(out=t_xw, in_=xw[:, :])
        t_ones = singles.tile([128, 16 * 32], mybir.dt.bfloat16)
        nc.sync.dma_start(out=t_ones, in_=onesw[:, :])
        t_odr = None
        if "sqdr" in skip:
            t_odr = singles.tile([128, 8, 2, 32], mybir.dt.float8e4)
            nc.sync.dma_start(out=t_odr, in_=onesdr[:, :, :, :])
        if "sqfp8" in skip:
            t_ones8 = singles.tile([128, 16 * 32], mybir.dt.float8e4)
            nc.sync.dma_start(out=t_ones8, in_=ones8[:, :])

        loop_ctx = tc.For_i(0, loop_iters, 1) if loop_iters > 1 else nullcontext()
        ctx.enter_context(loop_ctx)

        chunk_t = {}
        quad_ps = {}

        dma_eng = nc.gpsimd if "dmaswdge" in skip else nc.sync

        def finalize(q):
            dot_ps, sqn_ps = quad_ps.pop(q)
            if "fin" in skip:
                return
            sqv = finp.tile([128, s_sz], mybir.dt.float32, tag="fsq", name=f"fsq{q}")
            nc.scalar.activation(
                sqv, sqn_ps, mybir.ActivationFunctionType.Sqrt,
                scale=4.0 if normalize else 1.0,
            )
            rv = finp.tile([128, s_sz], mybir.dt.float32, tag="frv", name=f"frv{q}")
            nc.vector.reciprocal_approx_fast(out=rv, in_=sqv)
            simv = finp.tile([128, s_sz], mybir.dt.float32, tag="fsim", name=f"fsim{q}")
            nc.vector.tensor_mul(simv, dot_ps, rv)
            if normalize:
                nc.vector.tensor_scalar_add(simv, simv, 0.5)
            nc.sync.dma_start(out=out[q * 128:(q + 1) * 128, :], in_=simv)

        # c-innermost order: per l-step emit the 4 dot mms across the 4 PE
        # column groups back-to-back, then the 4 sqn mms — each matmul's
        # weight load hides behind the other column groups' streams.
        def flush_sqn(grp):
            if "mm" not in skip and "sqmm" not in skip:
                for (c, l, q, Sq, sqn_ps) in grp:
                    nc.tensor.matmul(
                        sqn_ps[32 * c:32 * (c + 1), :],
                        lhsT=t_ones[:, l * 32:(l + 1) * 32],
                        rhs=Sq,
                        start=(l == 0),
                        stop=(l == 15),
                        tile_position=(0, 32 * c),
                    )
            if grp and grp[-1][1] == 15:
                finalize(grp[-1][2])

        pend = None
        for stepi in range(NPp // 4):
            grp = []
            for i in range(4):
                pp = 4 * stepi + i
                q, r = pp // 64, pp % 64
                l, c = r // 4, r % 4
                ch = pp // CH
                if pp % CH == 0:
                    t = supp.tile([128, CH * s_sz], mybir.dt.bfloat16, tag="sup", name=f"sup{ch}")
                    dma_eng.dma_start(out=t, in_=sup[:, ch * CH * s_sz:(ch + 1) * CH * s_sz])
                    chunk_t[ch] = t
                if r == 0:
                    dot_ps = psDot.tile([128, s_sz], mybir.dt.float32, tag="dotq", name=f"dot{q}")
                    sqn_ps = psSqn.tile([128, s_sz], mybir.dt.float32, tag="sqnq", name=f"sqn{q}")
                    if "fin" not in skip:
                        if "mm" in skip or "dotmm" in skip:
                            nc.vector.memset(dot_ps, 0.0)
                        if "mm" in skip or "sqmm" in skip:
                            nc.vector.memset(sqn_ps, 1.0)
                    quad_ps[q] = (dot_ps, sqn_ps)
                dot_ps, sqn_ps = quad_ps[q]
                rhs = chunk_t[ch][:, (pp % CH) * s_sz:(pp % CH + 1) * s_sz]
                Sq = None
                if not ("sq" in skip and "mm" in skip):
                    Sq = sqp.tile([128, s_sz], mybir.dt.bfloat16, tag="sqt", name=f"sq{pp}")
                if "sq" not in skip:
                    if "sqdve" not in skip and pp % 2 == 1:
                        nc.scalar.activation(Sq, rhs, mybir.ActivationFunctionType.Square)
                    else:
                        nc.vector.tensor_mul(Sq, rhs, rhs)
                if "mm" not in skip and "dotmm" not in skip:
                    nc.tensor.matmul(
                        dot_ps[32 * c:32 * (c + 1), :],
                        lhsT=t_xw[:, pp * 32:(pp + 1) * 32],
                        rhs=rhs,
                        start=(l == 0),
                        stop=(l == 15),
                        tile_position=(0, 32 * c),
                    )
                grp.append((c, l, q, Sq, sqn_ps))
            if "sqlag" in skip:
                if pend is not None:
                    flush_sqn(pend)
                pend = grp
            else:
                flush_sqn(grp)
        if pend is not None:
            flush_sqn(pend)

    nc.finalize()
    return nc


def _pair_indices(bl_sz):
    """c-innermost pair order: consecutive matmuls hit different PE column
    groups, so their weight loads hide behind other groups' streams."""
    jp = np.arange(bl_sz // 2)
    q, r = jp // 64, jp % 64
    l, c = r // 4, r % 4
    b0 = q * 128 + 32 * c + 2 * l
    return b0, b0 + 1, l


def _pack_support(support_set):
    """Per-core pair-transposed bf16 pack: [128, NP*S]."""
    sup = np.asarray(support_set, np.float32)
    b0, b1, _ = _pair_indices(BL)
    cores = []
    for k in range(NCORES):
        sc = sup[:, k * BL:(k + 1) * BL, :]              # [S, BL, D]
        g0 = sc[:, b0, :].astype(BF16)                   # [S, NP, D]
        g1 = sc[:, b1, :].astype(BF16)
        # -> [2, D, NP, S] -> [128, NP*S]
        arr = np.stack([g0, g1], axis=0).transpose(0, 3, 2, 1)
        cores.append(np.ascontiguousarray(arr).reshape(128, NP * S))
    return cores


def _pack_host_inputs(x_hat, bl_sz):
    """Fold 1/max(||x||,eps) into X, pack per-core zero-padded bf16 lhsT mats."""
    x = np.asarray(x_hat, np.float32)
    xnorm = np.sqrt((x * x).sum(axis=1, keepdims=True))
    xn = (x / np.maximum(xnorm, 1e-10)).astype(BF16)

    ncores = x.shape[0] // bl_sz
    np_pairs = bl_sz // 2
    b0, b1, ls = _pair_indices(bl_sz)
    xw_cores = []
    for k in range(ncores):
        xw = np.zeros((128, np_pairs * 32), dtype=BF16)
        for jp in range(np_pairs):
            l = int(ls[jp])
            col = jp * 32
            xw[0:64, col + 2 * l] = xn[k * bl_sz + b0[jp]]
            xw[64:128, col + 2 * l + 1] = xn[k * bl_sz + b1[jp]]
        xw_cores.append(xw)

    onesw = np.zeros((128, 16 * 32), dtype=BF16)
    for l in range(16):
        onesw[0:64, l * 32 + 2 * l] = BF16(1.0)
        onesw[64:128, l * 32 + 2 * l + 1] = BF16(1.0)
    return xw_cores, onesw


FP8 = ml_dtypes.float8_e4m3


def _pack_ones_dr():
    """[Ki=128, t=8, Ko=2, M=32] fp8 weights: mm t covers pairs (l=2t, 2t+1);
    out rows 4t..4t+3 <- (ko=0, ki<64), (ko=0, ki>=64), (ko=1, ki<64), (ko=1, ki>=64)."""
    arr = np.zeros((128, 8, 2, 32), dtype=FP8)
    for t in range(8):
        arr[0:64, t, 0, 4 * t] = FP8(1.0)
        arr[64:128, t, 0, 4 * t + 1] = FP8(1.0)
        arr[0:64, t, 1, 4 * t + 2] = FP8(1.0)
        arr[64:128, t, 1, 4 * t + 3] = FP8(1.0)
    return arr


def _get_program(normalize):
    key = (S, BL, bool(normalize))
    if key not in _prog_cache:
        _prog_cache[key] = _build(S, BL, bool(normalize))
    return _prog_cache[key]


def _make_in_maps(support_set, X_hat):
    sup_cores = _pack_support(support_set)
    xw_cores, onesw = _pack_host_inputs(np.asarray(X_hat, np.float32), BL)
    onesdr = _pack_ones_dr()
    ones8 = onesw.astype(FP8)
    return [
        {"supT": sup_cores[k], "xw": xw_cores[k], "onesw": onesw,
         "onesdr": onesdr, "ones8": ones8}
        for k in range(NCORES)
    ]


def _run(support_set, X_hat, normalize, **spmd_kwargs):
    nrm = bool(np.asarray(normalize).item())
    from concourse.bass_utils import run_bass_kernel_spmd

    nc = _get_program(nrm)
    in_maps = _make_in_maps(support_set, X_hat)
    res = run_bass_kernel_spmd(nc, in_maps, list(range(NCORES)), **spmd_kwargs)
    out = np.empty((S, B), dtype=np.float32)
    for k in range(NCORES):
        out[:, k * BL:(k + 1) * BL] = np.asarray(res.results[k]["outT"]).T
    return out, res


def kernel(support_set, X_hat, normalize):
    out, _ = _run(support_set, X_hat, normalize)
    return out
